# revision 30
# baseline (speedup 1.0000x reference)
"""GCN-VAE (2-layer GCN encoder + reparameterization) on 8 Trainium2 cores.

Math: gcn_conv(x, W, b) = (segsum(x[src]*norm, dst) + x*dinv^2) @ W + b with
norm[e] = dinv[src]*dinv[dst].  Matmul commutes with the segment sum, so with
ts = (x @ W1) * dinv (a scaled table) the whole model is:

  L1: ts1 = (x @ W1) * dinv
  L2: hs  = relu(dinv*(segsum(ts1[src], dst) + ts1) + b1) * dinv
  L3: P2  = dinv*(segsum(hs[src], dst) + hs)
      z_mean = P2 @ W_mu + b_mu ; z_var = softplus(P2 @ W_var + b_var)
      z = z_mean + z_var * eps

(the mu and var branches share one propagation).

Distribution: nodes are globally sorted by in-degree and dealt round-robin to
the 8 cores, so every core has an (almost) identical degree profile and all
cores share ONE static schedule (SPMD).  Tables are bf16 [n_tab, 128] (64
real cols = 256B rows).  Per layer, each core:
  - dma_gather's its edges' source rows (dense 256B tokens, grouped by
    src-chunk of <32768 rows for the int16 indices, then by dst-block of 64
    nodes, runs padded to whole 128-token tiles with cross-core-common
    lengths).  Gather calls round-robin SWDGE queues 0-3: each queue is a
    different Q7 core pair, and pairs pipeline across instructions, so
    descriptor emission (the hard bottleneck, ~8ns/token on one pair) runs
    ~3.3x faster (~2.6ns/token measured),
  - segment-sums each 128-token tile into its dst block with one PE matmul
    against a one-hot indicator tile generated ON DEVICE by the vector
    engine (is_equal of a per-tile dst-slot code row vs an iota row; pad
    tokens carry -1 so their columns are all-zero, exact no-ops),
  - accumulates run partials in banked PSUM tiles [64, 8 runs, 64] and
    folds 8 runs per DVE op into the SBUF accumulator; epilogues are
    batched 8 node-blocks per instruction (L2 relu, L3 heads with
    per-oct softplus/reparameterization on ACT+DVE).
L1 runs the 512->64 GEMM in bf16 (fp32 PE matmuls are ~4x slower).  No
scatter is used anywhere (dma_scatter_add drops duplicate-index updates
on HW).  Between launches the host concatenates the 8 shard outputs into
the next full table replica (the "halo exchange").  Epilogues/heads are emitted
from a callback at each final-chunk PSUM fold so they interleave with the
remaining gather stream instead of serializing after it, and indicator
slabs are prefetched one group ahead so the PE never stalls behind
interleaved epilogue ops on the in-order DVE queue.  The node->core deal is
balanced per degree-rank block against per-(core, chunk) in-edge counts
over a slot-major table (see _permute), cutting schedule padding and
evening out run lengths.  Measured: 4.94ms -> 2.10ms total HW exec
(L1 78us, L2 968us, L3 1053us), rel err 1.7e-3.
"""

import sys
from contextlib import nullcontext

if "/opt/trn_rl_repo" not in sys.path:
    sys.path.insert(0, "/opt/trn_rl_repo")

import numpy as np

import concourse.bacc as bacc
import concourse.bass as bass
import concourse.mybir as mybir
import concourse.tile as tile
from concourse.bass_utils import run_bass_kernel_spmd
from concourse.masks import make_identity

M = 8  # number of NeuronCores
P = 128  # SBUF partitions
BM = 64  # dst nodes per indicator matmul (= feature width H)
F32 = mybir.dt.float32
BF16 = mybir.dt.bfloat16
I16 = mybir.dt.int16
AF = mybir.ActivationFunctionType

CALL_TOKENS = 4096  # max dma_gather tokens per call (HW-safe limit)
L1_MCHUNK = 14  # 128-node tiles per resident x-slab group in L1
HB = 128  # padded bf16 table row (64 real + 64 zero cols) = 256B
IND_G = 48  # indicator tiles per on-the-fly one-hot slab
NQ = 4  # SWDGE queues (distinct Q7 core pairs; gathers on different
#         queues overlap ~3.3x, measured 2.59ns/token vs 8.66 on one queue)

PROFILE = False  # set True (e.g. from test.py) to collect HW exec times
LAST_EXEC_NS = None  # sum over the three launches, max over cores
LAST_PER_LAUNCH = None


def _bf16_dtype():
    import ml_dtypes

    return ml_dtypes.bfloat16


# ----------------------------------------------------------------------------
# host-side preprocessing
# ----------------------------------------------------------------------------


def _permute(N, dst, src=None):
    """Global degree sort into 512-rank blocks (one 64-slot block per core),
    then balance node->core assignment WITHIN each block so per-(core, chunk)
    in-edge counts are as equal as possible.  The schedule pads every run to
    the cross-core max rounded to 128-token tiles, so imbalance costs whole
    tiles (t_tot 265k -> 255k tokens/core/layer when balanced).

    The gather table is laid out SLOT-MAJOR (gpos = slot*M + core), which
    makes an edge's chunk a function of the source's degree-rank block only
    -- invariant under the core re-assignment the balancer performs, so one
    greedy pass balances the true counts (core-major layout makes chunk
    depend on core and the passes oscillate)."""
    nsh = N // M
    nsh_pad = -(-nsh // P) * P
    indeg = np.bincount(dst, minlength=N)
    order = np.argsort(-indeg, kind="stable")  # rank -> node
    rank = np.empty(N, dtype=np.int64)
    rank[order] = np.arange(N)
    core_of = rank % M
    slot_of = rank // M

    if src is not None:
        spc = max(1, 32768 // nsh_pad)
        chunk_rows = min(spc * nsh_pad, M * nsh_pad)
        n_ch = -(-M * nsh_pad // chunk_rows)
        bpc = chunk_rows // (M * BM)  # rank-blocks per chunk
        BK = BM * M  # ranks per balance block (512)
        echunk = np.minimum((rank[src] // BK) // bpc, n_ch - 1)
        k = np.zeros((N, n_ch), dtype=np.int64)
        np.add.at(k, (dst, echunk), 1)
        core_of = np.empty(N, dtype=np.int64)
        for b0 in range(0, N, BK):
            nodes_b = order[b0 : b0 + BK]
            cap = len(nodes_b) // M
            kb = k[nodes_b]
            heavy = np.argsort(-kb.sum(axis=1), kind="stable")
            load = np.zeros((M, n_ch), dtype=np.int64)
            used = np.zeros(M, dtype=np.int64)
            for i in heavy:
                ki = kb[i]
                cand = load + ki
                score = cand.max(axis=1) * 1e6 + cand.sum(axis=1)
                score[used >= cap] = np.inf
                best = int(score.argmin())
                load[best] += ki
                used[best] += 1
                core_of[nodes_b[i]] = best
        slot_of = np.empty(N, dtype=np.int64)
        for b0 in range(0, N, BK):
            nodes_b = order[b0 : b0 + BK]
            s0 = (b0 // BK) * BM
            for m in range(M):
                mine = nodes_b[core_of[nodes_b] == m]
                slot_of[mine] = s0 + np.arange(len(mine))
        gpos = slot_of * M + core_of  # slot-major table
    else:
        gpos = core_of * nsh_pad + slot_of

    nodes = np.empty((M, nsh), dtype=np.int64)
    nodes[core_of[order], slot_of[order]] = order
    return nsh, nsh_pad, gpos, core_of, slot_of, nodes


def _schedule(src, dst, nsh_pad, gpos, core_of, slot_of):
    """Common token/matmul schedule + per-core idx & indicator arrays."""
    nblk = nsh_pad // P
    nb64 = nsh_pad // BM
    n_tab = M * nsh_pad
    shards_per_chunk = max(1, 32768 // nsh_pad)
    chunk_rows = min(shards_per_chunk * nsh_pad, n_tab)
    n_chunks = -(-n_tab // chunk_rows)

    ecore = core_of[dst]
    eblk = slot_of[dst] // BM
    echunk = gpos[src] // chunk_rows
    esrcrel = (gpos[src] % chunk_rows).astype(np.int64)

    key = (ecore * n_chunks + echunk) * nb64 + eblk
    cnt = np.bincount(key, minlength=M * n_chunks * nb64).reshape(
        M, n_chunks, nb64
    )
    runlen = cnt.max(axis=0)  # [n_chunks, nb64] common across cores
    runlen_pad = -(-runlen // P) * P  # whole 128-token tiles
    ntiles_run = runlen_pad // P

    run_off = np.zeros((n_chunks, nb64), dtype=np.int64)
    chunk_tok = np.zeros(n_chunks + 1, dtype=np.int64)
    t = 0
    for c in range(n_chunks):
        for b in range(nb64):
            run_off[c, b] = t
            t += int(runlen_pad[c, b])
        chunk_tok[c + 1] = t
    t_tot = t
    assert t_tot % 128 == 0 and t_tot > 0

    idx_rel = np.zeros((M, t_tot), dtype=np.int16)
    dst_loc = np.full((M, t_tot), -1, dtype=np.int16)  # -1 = pad token
    eord = np.argsort(key, kind="stable")
    ks = key[eord]
    ne = len(ks)
    grp_start = np.zeros(ne, dtype=np.int64)
    new_grp = np.ones(ne, dtype=bool)
    new_grp[1:] = ks[1:] != ks[:-1]
    starts = np.where(new_grp)[0]
    grp_start[starts] = starts
    grp_start = np.maximum.accumulate(grp_start)
    wpos = np.arange(ne) - grp_start
    e_core = ks // (n_chunks * nb64)
    e_chunk = (ks // nb64) % n_chunks
    e_blk = ks % nb64
    tok = run_off[e_chunk, e_blk] + wpos
    idx_rel[e_core, tok] = esrcrel[eord].astype(np.int16)
    dst_loc[e_core, tok] = (slot_of[dst][eord] % BM).astype(np.int16)

    # matmul schedule (common): one mm per 128-token tile
    mms = []  # (chunk, block64, tok0, start, stop)
    for c in range(n_chunks):
        for b in range(nb64):
            nt = int(ntiles_run[c, b])
            for k in range(nt):
                mms.append(
                    (c, b, int(run_off[c, b]) + k * P, k == 0, k == nt - 1)
                )
    n_mm = len(mms)

    # per-core dst-slot codes, partition-major [P, n_mm] bf16; the one-hot
    # indicator tiles are generated on-device via is_equal against an iota
    # row (pad tokens carry -1 and compare to nothing)
    assert (ntiles_run > 0).all()
    tok0s = np.array([t0 for (_, _, t0, _, _) in mms], dtype=np.int64)
    gathered = dst_loc[:, tok0s[:, None] + np.arange(P)[None, :]]  # [M,n_mm,P]
    dstloc_pm = np.ascontiguousarray(
        gathered.transpose(0, 2, 1).astype(_bf16_dtype())
    )
    iota_row = np.ascontiguousarray(
        np.broadcast_to(np.arange(BM, dtype=np.float32), (P, BM))
    ).astype(_bf16_dtype())
    jj = np.arange(t_tot)

    # wrapped int16 idx tiles: token j at [j%16, j//16], replicated x8
    wrapped = np.zeros((M, 16, t_tot // 16), dtype=np.int16)
    wrapped[:, jj % 16, jj // 16] = idx_rel
    wrapped = np.ascontiguousarray(np.tile(wrapped, (1, 8, 1)))

    # gather calls: per chunk, <= CALL_TOKENS multiples of 128
    calls = []  # (chunk, tok0, ntok)
    for c in range(n_chunks):
        a, end = int(chunk_tok[c]), int(chunk_tok[c + 1])
        while a < end:
            n = min(CALL_TOKENS, end - a)
            calls.append((c, a, n))
            a += n

    return dict(
        nblk=nblk, nb64=nb64, n_tab=n_tab, chunk_rows=chunk_rows,
        n_chunks=n_chunks, t_tot=t_tot, mms=mms, n_mm=n_mm, calls=calls,
        idx_wrapped=wrapped, dstloc_pm=dstloc_pm, iota_row=iota_row,
    )


# ----------------------------------------------------------------------------
# kernel builders
# ----------------------------------------------------------------------------


def _build_l1(I_DIM, nsh_pad, nblk, repeat=1):
    """ts1 = (x @ W1) * dinv as a bf16 [nsh_pad, 128] padded table shard."""
    nc = bacc.Bacc(None, target_bir_lowering=False)
    xT = nc.dram_tensor("xT", [I_DIM, nsh_pad], BF16, kind="ExternalInput")
    w1 = nc.dram_tensor("w1", [I_DIM, BM], BF16, kind="ExternalInput")
    dinv_cols = nc.dram_tensor("dinv_cols", [P, nblk], F32, kind="ExternalInput")
    out = nc.dram_tensor("ts1", [nsh_pad, HB], BF16, kind="ExternalOutput")
    kt = I_DIM // P

    with tile.TileContext(nc) as tc:
        with (
            tc.tile_pool(name="xslab", bufs=2) as xslab_tp,
            tc.tile_pool(name="const", bufs=1) as const_tp,
            tc.tile_pool(name="psum", bufs=8, space="PSUM") as psum_tp,
            tc.tile_pool(name="stage", bufs=1) as stage_tp,
            tc.For_i(0, repeat, 1) if repeat > 1 else nullcontext(),
        ):
            w1_raw = const_tp.tile([P, kt, BM], BF16)
            nc.sync.dma_start(
                out=w1_raw[:], in_=w1.rearrange("(k p) h -> p k h", p=P)
            )
            w1_s = const_tp.tile([P, kt, BM], BF16)
            nc.vector.tensor_copy(out=w1_s[:], in_=w1_raw[:])
            dinv_s = const_tp.tile([P, nblk], F32)
            nc.sync.dma_start(out=dinv_s[:], in_=dinv_cols[:, :])
            stage = stage_tp.tile([P, nblk, HB], BF16)
            nc.vector.memset(stage[:], 0.0)

            xT_r = xT.rearrange("(k p) m -> p k m", p=P)
            for c0 in range(0, nblk, L1_MCHUNK):
                mw = min(L1_MCHUNK, nblk - c0)
                slab = xslab_tp.tile([P, kt, L1_MCHUNK * P], BF16, tag="raw")
                nc.sync.dma_start(
                    out=slab[:, :, : mw * P],
                    in_=xT_r[:, :, c0 * P : (c0 + mw) * P],
                )
                for m in range(mw):
                    ps = psum_tp.tile([P, BM], F32, space="PSUM")
                    for k in range(kt):
                        nc.tensor.matmul(
                            ps[:],
                            lhsT=slab[:, k, m * P : (m + 1) * P],
                            rhs=w1_s[:, k, :],
                            start=(k == 0),
                            stop=(k == kt - 1),
                        )
                    b = c0 + m
                    nc.vector.tensor_scalar_mul(
                        out=stage[:, b, :BM], in0=ps[:],
                        scalar1=dinv_s[:, b : b + 1],
                    )
            nc.sync.dma_start(
                out=out.rearrange("(b p) h -> p b h", p=P), in_=stage[:]
            )
    nc.finalize()
    return nc


def _emit_prop(nc, sched, tabs, idx_s, ind_loader, agg, msg_tp, psum_tp,
               parts="gme", on_fold=None):
    """Gather calls + indicator matmuls + PSUM->SBUF folds into agg.

    on_fold(b0_64, n_64) fires right after the FINAL chunk's bank fold, so
    per-block epilogues can interleave with the remaining gather stream
    instead of serializing after it."""
    mms, calls = sched["mms"], sched["calls"]
    last_chunk = sched["n_chunks"] - 1
    call_bounds = [(ci, c, t0, n) for ci, (c, t0, n) in enumerate(calls)]
    msg_tiles = {}
    issued = set()

    def ensure_call(ci):
        if ci in issued:
            return
        issued.add(ci)
        _, c, t0, n = call_bounds[ci]
        q = ci % NQ
        mt = msg_tp.tile([P, CALL_TOKENS // P, HB], BF16, tag=f"msg{q}")
        msg_tiles[ci] = mt
        if "g" not in parts:
            return
        nc.gpsimd.dma_gather(
            mt[:, : n // P, :],
            tabs[c],
            idx_s[:, t0 // 16 : (t0 + n) // 16],
            n,
            n,
            HB,
            single_packet=False,
            queue_num=q,
        )

    def find_call(tok0):
        for ci, c, t0, n in call_bounds:
            if t0 <= tok0 < t0 + n:
                return ci, (tok0 - t0) // P
        raise AssertionError(tok0)

    if "m" not in parts:
        for ci in range(len(call_bounds)):
            ensure_call(ci)
        return
    bank = None  # PSUM tile [BM, 8, BM]: one slot per run (64-block)
    bank_b0 = None

    def flush_bank(n_in_bank):
        bb0 = bank_b0 // 2
        ne = (n_in_bank + 1) // 2
        no = n_in_bank // 2
        nc.vector.tensor_add(
            out=agg[:BM, bb0 : bb0 + ne, :],
            in0=agg[:BM, bb0 : bb0 + ne, :],
            in1=bank[:, 0:n_in_bank:2, :],
        )
        if no:
            nc.vector.tensor_add(
                out=agg[BM:, bb0 : bb0 + no, :],
                in0=agg[BM:, bb0 : bb0 + no, :],
                in1=bank[:, 1:n_in_bank:2, :],
            )

    for i, (c, b, t0, start, stop) in enumerate(mms):
        ci, slot = find_call(t0)
        ensure_call(ci)
        ind_tile = ind_loader(i)
        if start and b % 8 == 0:
            bank = psum_tp.tile([BM, 8, BM], F32, space="PSUM", tag="agg")
            bank_b0 = b
        nc.tensor.matmul(
            bank[:, b % 8, :],
            lhsT=ind_tile,
            rhs=msg_tiles[ci][:, slot, :BM],
            start=start,
            stop=stop,
        )
        if stop and (b % 8 == 7 or i + 1 == len(mms) or mms[i + 1][0] != c):
            flush_bank(b % 8 + 1)
            if on_fold is not None and c == last_chunk:
                on_fold(bank_b0, b % 8 + 1)


def _make_ind_loader(nc, dstloc_s, iota_s, ind_tp, n_mm):
    """Generate one-hot indicator slabs on the DVE: ind[p,i,c] =
    (dstloc[p,i] == c), batched IND_G tiles per tensor_tensor(is_equal).
    Slab g+1 is generated once g is half-consumed, so its DVE op lands
    ahead of any interleaved epilogue ops and the PE never stalls on it."""
    slabs = {}

    def gen(g):
        if g in slabs or g * IND_G >= n_mm:
            return
        lo, hi = g * IND_G, min((g + 1) * IND_G, n_mm)
        ng = hi - lo
        sl = ind_tp.tile([P, IND_G, BM], BF16, tag="islab")
        nc.vector.tensor_tensor(
            out=sl[:, :ng, :],
            in0=dstloc_s[:, lo:hi].unsqueeze(2).broadcast_to([P, ng, BM]),
            in1=iota_s[:].unsqueeze(1).broadcast_to([P, ng, BM]),
            op=mybir.AluOpType.is_equal,
        )
        slabs[g] = sl

    def loader(i):
        g = i // IND_G
        gen(g)
        if i % IND_G >= IND_G // 2:
            gen(g + 1)
        return slabs[g][:, i % IND_G, :]

    return loader


def _build_l2(sched, nsh_pad, nblk, has_b1, repeat=1, parts="gme"):
    n_tab, t_tot, n_mm = sched["n_tab"], sched["t_tot"], sched["n_mm"]
    chunk_rows = sched["chunk_rows"]
    nc = bacc.Bacc(None, target_bir_lowering=False, num_swdge_queues=NQ)
    tab = nc.dram_tensor("tab", [n_tab, HB], BF16, kind="ExternalInput")
    own = nc.dram_tensor("own", [nsh_pad, HB], BF16, kind="ExternalInput")
    idx = nc.dram_tensor("idx", [P, t_tot // 16], I16, kind="ExternalInput")
    dstloc = nc.dram_tensor("dstloc", [P, n_mm], BF16, kind="ExternalInput")
    iota = nc.dram_tensor("iota", [P, BM], BF16, kind="ExternalInput")
    dinv_cols = nc.dram_tensor("dinv_cols", [P, nblk], F32, kind="ExternalInput")
    if has_b1:
        b1bc = nc.dram_tensor("b1bc", [P, BM], F32, kind="ExternalInput")
    out = nc.dram_tensor("hs", [nsh_pad, HB], BF16, kind="ExternalOutput")

    with tile.TileContext(nc) as tc:
        with (
            tc.tile_pool(name="const", bufs=1) as const_tp,
            tc.tile_pool(name="msg", bufs=3) as msg_tp,
            tc.tile_pool(name="indp", bufs=3) as ind_tp,
            tc.tile_pool(name="psum", bufs=4, space="PSUM") as psum_tp,
            tc.tile_pool(name="stage", bufs=1) as stage_tp,
            tc.For_i(0, repeat, 1) if repeat > 1 else nullcontext(),
        ):
            idx_s = const_tp.tile([P, t_tot // 16], I16)
            nc.sync.dma_start(out=idx_s[:], in_=idx[:, :])
            own_s = const_tp.tile([P, nblk, BM], BF16)
            nc.sync.dma_start(
                out=own_s[:],
                in_=own.rearrange("(b p) h -> p b h", p=P)[:, :, :BM],
            )
            dinv_s = const_tp.tile([P, nblk], F32)
            nc.sync.dma_start(out=dinv_s[:], in_=dinv_cols[:, :])
            dsq = const_tp.tile([P, nblk], F32)
            nc.vector.tensor_mul(out=dsq[:], in0=dinv_s[:], in1=dinv_s[:])
            if has_b1:
                b1_s = const_tp.tile([P, BM], F32)
                nc.sync.dma_start(out=b1_s[:], in_=b1bc[:, :])
            dstloc_s = const_tp.tile([P, n_mm], BF16)
            nc.sync.dma_start(out=dstloc_s[:], in_=dstloc[:, :])
            iota_s = const_tp.tile([P, BM], BF16)
            nc.sync.dma_start(out=iota_s[:], in_=iota[:, :])
            agg = stage_tp.tile([P, nblk, BM], F32, tag="agg")
            nc.vector.memset(agg[:], 0.0)
            stage = stage_tp.tile([P, nblk, BM], BF16, tag="out")

            loader = _make_ind_loader(nc, dstloc_s, iota_s, ind_tp, n_mm)
            tabs = {
                c: tab[c * chunk_rows : min((c + 1) * chunk_rows, n_tab), :]
                for c in range(sched["n_chunks"])
            }
            def epi(b064, n64):
                # hs = relu(agg*dinv)*dinv = relu(agg*dinv^2), 4 blocks/op
                b0, nb = b064 // 2, n64 // 2
                B = slice(b0, b0 + nb)
                nc.vector.tensor_add(
                    out=agg[:, B, :], in0=agg[:, B, :], in1=own_s[:, B, :]
                )
                nc.vector.tensor_mul(
                    out=agg[:, B, :], in0=agg[:, B, :],
                    in1=dsq[:, B].unsqueeze(2).broadcast_to([P, nb, BM]),
                )
                nc.scalar.activation(
                    out=stage[:, B, :], in_=agg[:, B, :], func=AF.Relu
                )

            use_cb = (not has_b1) and "e" in parts
            _emit_prop(nc, sched, tabs, idx_s, loader, agg, msg_tp, psum_tp,
                       parts=parts, on_fold=epi if use_cb else None)

            if has_b1 and "e" in parts:
                for b in range(nblk):
                    nc.vector.tensor_add(
                        out=agg[:, b, :], in0=agg[:, b, :], in1=own_s[:, b, :]
                    )
                    # hs = relu(agg*dinv + b1)*dinv; relu(y)*d = relu(y*d), d>0
                    nc.vector.tensor_scalar_mul(
                        out=agg[:, b, :], in0=agg[:, b, :],
                        scalar1=dinv_s[:, b : b + 1],
                    )
                    nc.vector.tensor_add(
                        out=agg[:, b, :], in0=agg[:, b, :], in1=b1_s[:]
                    )
                    nc.scalar.activation(
                        out=stage[:, b, :], in_=agg[:, b, :], func=AF.Relu,
                        scale=dinv_s[:, b : b + 1],
                    )
            nc.sync.dma_start(
                out=out.rearrange("(b p) h -> p b h", p=P)[:, :, :BM],
                in_=stage[:],
            )
    nc.finalize()
    return nc


def _build_l3(sched, nsh_pad, nblk, has_bmu, has_bvar, repeat=1):
    n_tab, t_tot, n_mm = sched["n_tab"], sched["t_tot"], sched["n_mm"]
    chunk_rows = sched["chunk_rows"]
    nc = bacc.Bacc(None, target_bir_lowering=False, num_swdge_queues=NQ)
    tab = nc.dram_tensor("tab", [n_tab, HB], BF16, kind="ExternalInput")
    own = nc.dram_tensor("own", [nsh_pad, HB], BF16, kind="ExternalInput")
    idx = nc.dram_tensor("idx", [P, t_tot // 16], I16, kind="ExternalInput")
    dstloc = nc.dram_tensor("dstloc", [P, n_mm], BF16, kind="ExternalInput")
    iota = nc.dram_tensor("iota", [P, BM], BF16, kind="ExternalInput")
    dinv_cols = nc.dram_tensor("dinv_cols", [P, nblk], F32, kind="ExternalInput")
    wmu = nc.dram_tensor("wmu", [BM, BM], F32, kind="ExternalInput")
    wvar = nc.dram_tensor("wvar", [BM, BM], F32, kind="ExternalInput")
    eps_sh = nc.dram_tensor("eps_sh", [nsh_pad, BM], BF16, kind="ExternalInput")
    if has_bmu:
        bmubc = nc.dram_tensor("bmubc", [P, BM], F32, kind="ExternalInput")
    if has_bvar:
        bvarbc = nc.dram_tensor("bvarbc", [P, BM], F32, kind="ExternalInput")
    zm = nc.dram_tensor("zm", [nsh_pad, BM], F32, kind="ExternalOutput")
    zv = nc.dram_tensor("zv", [nsh_pad, BM], F32, kind="ExternalOutput")
    zz = nc.dram_tensor("zz", [nsh_pad, BM], F32, kind="ExternalOutput")

    with tile.TileContext(nc) as tc:
        with (
            tc.tile_pool(name="const", bufs=1) as const_tp,
            tc.tile_pool(name="msg", bufs=2) as msg_tp,
            tc.tile_pool(name="indp", bufs=3) as ind_tp,
            tc.tile_pool(name="work", bufs=3) as work_tp,
            tc.tile_pool(name="psum", bufs=2, space="PSUM") as psum_tp,
            tc.tile_pool(name="stage", bufs=1) as stage_tp,
            tc.For_i(0, repeat, 1) if repeat > 1 else nullcontext(),
        ):
            idx_s = const_tp.tile([P, t_tot // 16], I16)
            nc.sync.dma_start(out=idx_s[:], in_=idx[:, :])
            own_s = const_tp.tile([P, nblk, BM], BF16)
            nc.sync.dma_start(
                out=own_s[:],
                in_=own.rearrange("(b p) h -> p b h", p=P)[:, :, :BM],
            )
            dinv_s = const_tp.tile([P, nblk], F32)
            nc.sync.dma_start(out=dinv_s[:], in_=dinv_cols[:, :])
            eps_s = const_tp.tile([P, nblk, BM], BF16)
            nc.sync.dma_start(
                out=eps_s[:], in_=eps_sh.rearrange("(b p) h -> p b h", p=P)
            )
            w_raw = const_tp.tile([P, 2, BM], F32)
            nc.sync.dma_start(out=w_raw[:BM, 0, :], in_=wmu[:, :])
            nc.sync.dma_start(out=w_raw[:BM, 1, :], in_=wvar[:, :])
            nc.sync.dma_start(out=w_raw[BM:, 0, :], in_=wmu[:, :])
            nc.sync.dma_start(out=w_raw[BM:, 1, :], in_=wvar[:, :])
            wmu_s = const_tp.tile([P, BM], F32)
            nc.vector.tensor_copy(out=wmu_s[:], in_=w_raw[:, 0, :])
            wvar_s = const_tp.tile([P, BM], F32)
            nc.vector.tensor_copy(out=wvar_s[:], in_=w_raw[:, 1, :])
            dstloc_s = const_tp.tile([P, n_mm], BF16)
            nc.sync.dma_start(out=dstloc_s[:], in_=dstloc[:, :])
            iota_s = const_tp.tile([P, BM], BF16)
            nc.sync.dma_start(out=iota_s[:], in_=iota[:, :])
            ident_raw = const_tp.tile([P, P], F32)
            make_identity(nc, ident_raw[:])
            ident = const_tp.tile([P, P], F32)
            nc.vector.tensor_copy(out=ident[:], in_=ident_raw[:])
            if has_bmu:
                bmu_s = const_tp.tile([P, BM], F32)
                nc.sync.dma_start(out=bmu_s[:], in_=bmubc[:, :])
            if has_bvar:
                bvar_s = const_tp.tile([P, BM], F32)
                nc.sync.dma_start(out=bvar_s[:], in_=bvarbc[:, :])
            agg = stage_tp.tile([P, nblk, BM], F32, tag="agg")
            nc.vector.memset(agg[:], 0.0)

            loader = _make_ind_loader(nc, dstloc_s, iota_s, ind_tp, n_mm)
            tabs = {
                c: tab[c * chunk_rows : min((c + 1) * chunk_rows, n_tab), :]
                for c in range(sched["n_chunks"])
            }
            zm_r = zm.rearrange("(b p) h -> p b h", p=P)
            zv_r = zv.rearrange("(b p) h -> p b h", p=P)
            zz_r = zz.rearrange("(b p) h -> p b h", p=P)

            def head(b064, n64):
                # heads for the 128-blocks of one fold bank (<=4 blocks),
                # emitted inside the gather stream so they overlap it
                g0, nb = b064 // 2, n64 // 2
                pmv = psum_tp.tile(
                    [P, 4, 2, BM], F32, space="PSUM", tag="pmv"
                )
                psm = pmv[:, :, 0, :]
                psv = pmv[:, :, 1, :]
                for j2 in range(nb // 2):
                    bA = g0 + 2 * j2
                    B = slice(bA, bA + 2)
                    nc.vector.tensor_add(
                        out=agg[:, B, :], in0=agg[:, B, :], in1=own_s[:, B, :]
                    )
                    p2 = work_tp.tile([P, 2, BM], F32, tag="p2")
                    nc.vector.tensor_mul(
                        out=p2[:], in0=agg[:, B, :],
                        in1=dinv_s[:, B].unsqueeze(2).broadcast_to([P, 2, BM]),
                    )
                    for k in range(2):
                        slot = 2 * j2 + k
                        pst = psum_tp.tile(
                            [BM, P], F32, space="PSUM", tag="pst"
                        )
                        nc.tensor.transpose(
                            out=pst[:], in_=p2[:, k, :], identity=ident[:]
                        )
                        p2t = work_tp.tile([BM, P], F32, tag="p2t")
                        nc.vector.tensor_copy(out=p2t[:], in_=pst[:])
                        # one wide matmul: rhs = [wmu | wvar] adjacent
                        # in w_raw rows 0:64 -> out [128, mu 64 | var 64]
                        nc.tensor.matmul(
                            pmv[:, slot, :, :],
                            lhsT=p2t[:],
                            rhs=w_raw[:BM, :, :].rearrange(
                                "p a b -> p (a b)"
                            ),
                            start=True,
                            stop=True,
                        )
                S = slice(g0, g0 + nb)
                sp = work_tp.tile([P, 4, BM], F32, tag="sp")
                if has_bvar:
                    nc.vector.tensor_add(
                        out=psv[:, :nb, :], in0=psv[:, :nb, :],
                        in1=bvar_s[:].unsqueeze(1).broadcast_to([P, nb, BM]),
                    )
                # softplus(u) = relu(u) + ln(1 + exp(-|u|))
                nc.scalar.activation(
                    out=sp[:, :nb, :], in_=psv[:, :nb, :], func=AF.Abs
                )
                nc.scalar.activation(
                    out=sp[:, :nb, :], in_=sp[:, :nb, :], func=AF.Exp,
                    scale=-1.0,
                )
                nc.scalar.activation(
                    out=sp[:, :nb, :], in_=sp[:, :nb, :], func=AF.Ln, bias=1.0
                )
                zv_t = work_tp.tile([P, 4, BM], F32, tag="zv_t")
                nc.vector.tensor_scalar_max(
                    out=zv_t[:, :nb, :], in0=psv[:, :nb, :], scalar1=0.0
                )
                nc.vector.tensor_add(
                    out=zv_t[:, :nb, :], in0=zv_t[:, :nb, :], in1=sp[:, :nb, :]
                )
                zm_t = work_tp.tile([P, 4, BM], F32, tag="zm_t")
                nc.scalar.activation(
                    out=zm_t[:, :nb, :], in_=psm[:, :nb, :], func=AF.Copy
                )
                if has_bmu:
                    nc.vector.tensor_add(
                        out=zm_t[:, :nb, :], in0=zm_t[:, :nb, :],
                        in1=bmu_s[:].unsqueeze(1).broadcast_to([P, nb, BM]),
                    )
                zz_t = work_tp.tile([P, 4, BM], F32, tag="zz_t")
                nc.vector.tensor_mul(
                    out=zz_t[:, :nb, :], in0=zv_t[:, :nb, :], in1=eps_s[:, S, :]
                )
                nc.vector.tensor_add(
                    out=zz_t[:, :nb, :], in0=zz_t[:, :nb, :], in1=zm_t[:, :nb, :]
                )
                nc.sync.dma_start(out=zm_r[:, S, :], in_=zm_t[:, :nb, :])
                nc.sync.dma_start(out=zv_r[:, S, :], in_=zv_t[:, :nb, :])
                nc.sync.dma_start(out=zz_r[:, S, :], in_=zz_t[:, :nb, :])

            _emit_prop(nc, sched, tabs, idx_s, loader, agg, msg_tp, psum_tp,
                       on_fold=head)
    nc.finalize()
    return nc


# ----------------------------------------------------------------------------
# top-level entry
# ----------------------------------------------------------------------------


def kernel(x, edge_index, W1, b1, W_mu, b_mu, W_var, b_var, eps):
    x = np.ascontiguousarray(np.asarray(x, dtype=np.float32))
    W1 = np.ascontiguousarray(np.asarray(W1, dtype=np.float32))
    W_mu = np.ascontiguousarray(np.asarray(W_mu, dtype=np.float32))
    W_var = np.ascontiguousarray(np.asarray(W_var, dtype=np.float32))
    b1 = np.asarray(b1, dtype=np.float32)
    b_mu = np.asarray(b_mu, dtype=np.float32)
    b_var = np.asarray(b_var, dtype=np.float32)
    eps = np.asarray(eps, dtype=np.float32)
    ei = np.asarray(edge_index, dtype=np.int64)

    N, I_DIM = x.shape
    H = W1.shape[1]
    assert N % M == 0 and I_DIM % P == 0 and H == BM

    src, dst = ei[0], ei[1]
    deg = (np.bincount(dst, minlength=N) + 1.0).astype(np.float32)
    dinv = (1.0 / np.sqrt(deg)).astype(np.float32)

    nsh, nsh_pad, gpos, core_of, slot_of, nodes = _permute(N, dst, src)
    sched = _schedule(src, dst, nsh_pad, gpos, core_of, slot_of)
    nblk = sched["nblk"]

    has_b1 = bool(np.any(b1 != 0))
    has_bmu = bool(np.any(b_mu != 0))
    has_bvar = bool(np.any(b_var != 0))

    xT_c, dinv_cols_c, eps_c = [], [], []
    for c in range(M):
        nl = nodes[c]
        xs = np.zeros((nsh_pad, I_DIM), dtype=np.float32)
        xs[:nsh] = x[nl]
        xT_c.append(np.ascontiguousarray(xs.T.astype(_bf16_dtype())))
        d = np.ones(nsh_pad, dtype=np.float32)
        d[:nsh] = dinv[nl]
        dinv_cols_c.append(np.ascontiguousarray(d.reshape(nblk, P).T))
        es = np.zeros((nsh_pad, H), dtype=np.float32)
        es[:nsh] = eps[nl]
        eps_c.append(es.astype(_bf16_dtype()))

    core_ids = list(range(M))
    exec_ns = []

    def _run(nc, in_maps):
        r = run_bass_kernel_spmd(nc, in_maps, core_ids, trace=PROFILE)
        if PROFILE:
            exec_ns.append(r.exec_time_ns)
        return r.results

    # ---- L1 ----
    nc1 = _build_l1(I_DIM, nsh_pad, nblk)
    W1_bf = W1.astype(_bf16_dtype())
    r1 = _run(
        nc1,
        [{"xT": xT_c[c], "w1": W1_bf, "dinv_cols": dinv_cols_c[c]} for c in range(M)],
    )
    ts1_c = [np.asarray(r1[c]["ts1"]) for c in range(M)]
    tab1 = np.ascontiguousarray(
        np.stack(ts1_c, axis=1).reshape(M * nsh_pad, HB)
    )

    # ---- L2 ----
    nc2 = _build_l2(sched, nsh_pad, nblk, has_b1)
    b1bc = np.broadcast_to(b1, (P, H)).copy() if has_b1 else None
    in_maps = []
    for c in range(M):
        im = {
            "tab": tab1,
            "own": ts1_c[c],
            "idx": sched["idx_wrapped"][c],
            "dstloc": sched["dstloc_pm"][c],
            "iota": sched["iota_row"],
            "dinv_cols": dinv_cols_c[c],
        }
        if has_b1:
            im["b1bc"] = b1bc
        in_maps.append(im)
    r2 = _run(nc2, in_maps)
    hs_c = [np.asarray(r2[c]["hs"]) for c in range(M)]
    tab2 = np.ascontiguousarray(
        np.stack(hs_c, axis=1).reshape(M * nsh_pad, HB)
    )

    # ---- L3 ----
    nc3 = _build_l3(sched, nsh_pad, nblk, has_bmu, has_bvar)
    bmubc = np.broadcast_to(b_mu, (P, H)).copy() if has_bmu else None
    bvarbc = np.broadcast_to(b_var, (P, H)).copy() if has_bvar else None
    in_maps = []
    for c in range(M):
        im = {
            "tab": tab2,
            "own": hs_c[c],
            "idx": sched["idx_wrapped"][c],
            "dstloc": sched["dstloc_pm"][c],
            "iota": sched["iota_row"],
            "dinv_cols": dinv_cols_c[c],
            "wmu": W_mu,
            "wvar": W_var,
            "eps_sh": eps_c[c],
        }
        if has_bmu:
            im["bmubc"] = bmubc
        if has_bvar:
            im["bvarbc"] = bvarbc
        in_maps.append(im)
    r3 = _run(nc3, in_maps)

    global LAST_EXEC_NS, LAST_PER_LAUNCH
    if PROFILE:
        LAST_PER_LAUNCH = exec_ns
        LAST_EXEC_NS = sum(t for t in exec_ns if t) if any(exec_ns) else None

    z_mean = np.empty((N, H), dtype=np.float32)
    z_var = np.empty((N, H), dtype=np.float32)
    z = np.empty((N, H), dtype=np.float32)
    for c in range(M):
        nl = nodes[c]
        z_mean[nl] = np.asarray(r3[c]["zm"])[:nsh]
        z_var[nl] = np.asarray(r3[c]["zv"])[:nsh]
        z[nl] = np.asarray(r3[c]["zz"])[:nsh]
    return z_mean, z_var, z



# revision 31
# speedup vs baseline: 1.1290x; 1.1290x over previous
"""GCN-VAE (2-layer GCN encoder + reparameterization) on 8 Trainium2 cores.

Math: gcn_conv(x, W, b) = (segsum(x[src]*norm, dst) + x*dinv^2) @ W + b with
norm[e] = dinv[src]*dinv[dst].  Matmul commutes with the segment sum, so with
ts = (x @ W1) * dinv (a scaled table) the whole model is:

  L1: ts1 = (x @ W1) * dinv
  L2: hs  = relu(dinv*(segsum(ts1[src], dst) + ts1) + b1) * dinv
  L3: P2  = dinv*(segsum(hs[src], dst) + hs)
      z_mean = P2 @ W_mu + b_mu ; z_var = softplus(P2 @ W_var + b_var)
      z = z_mean + z_var * eps

(the mu and var branches share one propagation).

Distribution: nodes are globally sorted by in-degree and dealt round-robin to
the 8 cores, so every core has an (almost) identical degree profile and all
cores share ONE static schedule (SPMD).  Tables are bf16 [n_tab, 128] (64
real cols = 256B rows).  Per layer, each core:
  - dma_gather's its edges' source rows (dense 256B tokens, grouped by
    src-chunk of <32768 rows for the int16 indices, then by dst-block of 64
    nodes, runs padded to whole 128-token tiles with cross-core-common
    lengths).  Gather calls round-robin SWDGE queues 0-3: each queue is a
    different Q7 core pair, and pairs pipeline across instructions, so
    descriptor emission (the hard bottleneck, ~8ns/token on one pair) runs
    ~3.3x faster (~2.6ns/token measured),
  - segment-sums each 128-token tile into its dst block with one PE matmul
    against a one-hot indicator tile generated ON DEVICE by the vector
    engine (is_equal of a per-tile dst-slot code row vs an iota row; pad
    tokens carry -1 so their columns are all-zero, exact no-ops),
  - accumulates run partials in banked PSUM tiles [64, 8 runs, 64] and
    folds 8 runs per DVE op into the SBUF accumulator; epilogues are
    batched 8 node-blocks per instruction (L2 relu, L3 heads with
    per-oct softplus/reparameterization on ACT+DVE).
L1 runs the 512->64 GEMM in bf16 (fp32 PE matmuls are ~4x slower).  No
scatter is used anywhere (dma_scatter_add drops duplicate-index updates
on HW).  Between launches the host concatenates the 8 shard outputs into
the next full table replica (the "halo exchange").  Epilogues/heads are emitted
from a callback at each final-chunk PSUM fold so they interleave with the
remaining gather stream instead of serializing after it, and indicator
slabs are prefetched one group ahead so the PE never stalls behind
interleaved epilogue ops on the in-order DVE queue.  The node->core deal is
balanced per degree-rank block against per-(core, chunk) in-edge counts
over a slot-major table (see _permute), cutting schedule padding and
evening out run lengths.  Measured: 4.94ms -> 2.10ms total HW exec
(L1 78us, L2 968us, L3 1053us), rel err 1.7e-3.
"""

import sys
from contextlib import nullcontext

if "/opt/trn_rl_repo" not in sys.path:
    sys.path.insert(0, "/opt/trn_rl_repo")

import numpy as np

import concourse.bacc as bacc
import concourse.bass as bass
import concourse.mybir as mybir
import concourse.tile as tile
from concourse.bass_utils import run_bass_kernel_spmd
from concourse.masks import make_identity

M = 8  # number of NeuronCores
P = 128  # SBUF partitions
BM = 64  # dst nodes per indicator matmul (= feature width H)
F32 = mybir.dt.float32
BF16 = mybir.dt.bfloat16
I16 = mybir.dt.int16
AF = mybir.ActivationFunctionType

CALL_TOKENS = 4096  # max dma_gather tokens per call (HW-safe limit)
L1_MCHUNK = 14  # 128-node tiles per resident x-slab group in L1
HB = 128  # padded bf16 table row (64 real + 64 zero cols) = 256B
IND_G = 48  # indicator tiles per on-the-fly one-hot slab
NQ = 4  # SWDGE queues (distinct Q7 core pairs; gathers on different
#         queues overlap ~3.3x, measured 2.59ns/token vs 8.66 on one queue)

PROFILE = False  # set True (e.g. from test.py) to collect HW exec times
LAST_EXEC_NS = None  # sum over the three launches, max over cores
LAST_PER_LAUNCH = None


def _bf16_dtype():
    import ml_dtypes

    return ml_dtypes.bfloat16


# ----------------------------------------------------------------------------
# host-side preprocessing
# ----------------------------------------------------------------------------


def _permute(N, dst, src=None):
    """Global degree sort into 512-rank blocks (one 64-slot block per core),
    then balance node->core assignment WITHIN each block so per-(core, chunk)
    in-edge counts are as equal as possible.  The schedule pads every run to
    the cross-core max rounded to 128-token tiles, so imbalance costs whole
    tiles (t_tot 265k -> 255k tokens/core/layer when balanced).

    The gather table is laid out SLOT-MAJOR (gpos = slot*M + core), which
    makes an edge's chunk a function of the source's degree-rank block only
    -- invariant under the core re-assignment the balancer performs, so one
    greedy pass balances the true counts (core-major layout makes chunk
    depend on core and the passes oscillate)."""
    nsh = N // M
    nsh_pad = -(-nsh // P) * P
    indeg = np.bincount(dst, minlength=N)
    order = np.argsort(-indeg, kind="stable")  # rank -> node
    rank = np.empty(N, dtype=np.int64)
    rank[order] = np.arange(N)
    core_of = rank % M
    slot_of = rank // M

    if src is not None:
        spc = max(1, 32768 // nsh_pad)
        chunk_rows = min(spc * nsh_pad, M * nsh_pad)
        n_ch = -(-M * nsh_pad // chunk_rows)
        bpc = chunk_rows // (M * BM)  # rank-blocks per chunk
        BK = BM * M  # ranks per balance block (512)
        echunk = np.minimum((rank[src] // BK) // bpc, n_ch - 1)
        k = np.zeros((N, n_ch), dtype=np.int64)
        np.add.at(k, (dst, echunk), 1)
        core_of = np.empty(N, dtype=np.int64)
        for b0 in range(0, N, BK):
            nodes_b = order[b0 : b0 + BK]
            cap = len(nodes_b) // M
            kb = k[nodes_b]
            heavy = np.argsort(-kb.sum(axis=1), kind="stable")
            load = np.zeros((M, n_ch), dtype=np.int64)
            used = np.zeros(M, dtype=np.int64)
            for i in heavy:
                ki = kb[i]
                cand = load + ki
                score = cand.max(axis=1) * 1e6 + cand.sum(axis=1)
                score[used >= cap] = np.inf
                best = int(score.argmin())
                load[best] += ki
                used[best] += 1
                core_of[nodes_b[i]] = best
        slot_of = np.empty(N, dtype=np.int64)
        for b0 in range(0, N, BK):
            nodes_b = order[b0 : b0 + BK]
            s0 = (b0 // BK) * BM
            for m in range(M):
                mine = nodes_b[core_of[nodes_b] == m]
                slot_of[mine] = s0 + np.arange(len(mine))
        gpos = slot_of * M + core_of  # slot-major table
    else:
        gpos = core_of * nsh_pad + slot_of

    nodes = np.empty((M, nsh), dtype=np.int64)
    nodes[core_of[order], slot_of[order]] = order
    return nsh, nsh_pad, gpos, core_of, slot_of, nodes


def _schedule(src, dst, nsh_pad, gpos, core_of, slot_of):
    """Common token/matmul schedule + per-core idx & indicator arrays."""
    nblk = nsh_pad // P
    nb64 = nsh_pad // BM
    n_tab = M * nsh_pad
    shards_per_chunk = max(1, 32768 // nsh_pad)
    chunk_rows = min(shards_per_chunk * nsh_pad, n_tab)
    n_chunks = -(-n_tab // chunk_rows)

    ecore = core_of[dst]
    eblk = slot_of[dst] // BM
    echunk = gpos[src] // chunk_rows
    esrcrel = (gpos[src] % chunk_rows).astype(np.int64)

    key = (ecore * n_chunks + echunk) * nb64 + eblk
    cnt = np.bincount(key, minlength=M * n_chunks * nb64).reshape(
        M, n_chunks, nb64
    )
    runlen = cnt.max(axis=0)  # [n_chunks, nb64] common across cores
    runlen_pad = -(-runlen // P) * P  # whole 128-token tiles
    ntiles_run = runlen_pad // P

    run_off = np.zeros((n_chunks, nb64), dtype=np.int64)
    chunk_tok = np.zeros(n_chunks + 1, dtype=np.int64)
    t = 0
    for c in range(n_chunks):
        for b in range(nb64):
            run_off[c, b] = t
            t += int(runlen_pad[c, b])
        chunk_tok[c + 1] = t
    t_tot = t
    assert t_tot % 128 == 0 and t_tot > 0

    idx_rel = np.zeros((M, t_tot), dtype=np.int16)
    dst_loc = np.full((M, t_tot), -1, dtype=np.int16)  # -1 = pad token
    eord = np.argsort(key, kind="stable")
    ks = key[eord]
    ne = len(ks)
    grp_start = np.zeros(ne, dtype=np.int64)
    new_grp = np.ones(ne, dtype=bool)
    new_grp[1:] = ks[1:] != ks[:-1]
    starts = np.where(new_grp)[0]
    grp_start[starts] = starts
    grp_start = np.maximum.accumulate(grp_start)
    wpos = np.arange(ne) - grp_start
    e_core = ks // (n_chunks * nb64)
    e_chunk = (ks // nb64) % n_chunks
    e_blk = ks % nb64
    tok = run_off[e_chunk, e_blk] + wpos
    idx_rel[e_core, tok] = esrcrel[eord].astype(np.int16)
    dst_loc[e_core, tok] = (slot_of[dst][eord] % BM).astype(np.int16)

    # matmul schedule (common): one mm per 128-token tile
    mms = []  # (chunk, block64, tok0, start, stop)
    for c in range(n_chunks):
        for b in range(nb64):
            nt = int(ntiles_run[c, b])
            for k in range(nt):
                mms.append(
                    (c, b, int(run_off[c, b]) + k * P, k == 0, k == nt - 1)
                )
    n_mm = len(mms)

    # per-core dst-slot codes, partition-major [P, n_mm] bf16; the one-hot
    # indicator tiles are generated on-device via is_equal against an iota
    # row (pad tokens carry -1 and compare to nothing)
    assert (ntiles_run > 0).all()
    tok0s = np.array([t0 for (_, _, t0, _, _) in mms], dtype=np.int64)
    gathered = dst_loc[:, tok0s[:, None] + np.arange(P)[None, :]]  # [M,n_mm,P]
    dstloc_pm = np.ascontiguousarray(
        gathered.transpose(0, 2, 1).astype(_bf16_dtype())
    )
    iota_row = np.ascontiguousarray(
        np.broadcast_to(np.arange(BM, dtype=np.float32), (P, BM))
    ).astype(_bf16_dtype())
    jj = np.arange(t_tot)

    # wrapped int16 idx tiles: token j at [j%16, j//16], replicated x8
    wrapped = np.zeros((M, 16, t_tot // 16), dtype=np.int16)
    wrapped[:, jj % 16, jj // 16] = idx_rel
    wrapped = np.ascontiguousarray(np.tile(wrapped, (1, 8, 1)))

    # gather calls: per chunk, <= CALL_TOKENS multiples of 128
    calls = []  # (chunk, tok0, ntok)
    for c in range(n_chunks):
        a, end = int(chunk_tok[c]), int(chunk_tok[c + 1])
        while a < end:
            n = min(CALL_TOKENS, end - a)
            calls.append((c, a, n))
            a += n

    return dict(
        nblk=nblk, nb64=nb64, n_tab=n_tab, chunk_rows=chunk_rows,
        n_chunks=n_chunks, t_tot=t_tot, mms=mms, n_mm=n_mm, calls=calls,
        idx_wrapped=wrapped, dstloc_pm=dstloc_pm, iota_row=iota_row,
    )


# ----------------------------------------------------------------------------
# kernel builders
# ----------------------------------------------------------------------------


def _build_l1(I_DIM, nsh_pad, nblk, repeat=1):
    """ts1 = (x @ W1) * dinv as a bf16 [nsh_pad, 128] padded table shard."""
    nc = bacc.Bacc(None, target_bir_lowering=False)
    xT = nc.dram_tensor("xT", [I_DIM, nsh_pad], BF16, kind="ExternalInput")
    w1 = nc.dram_tensor("w1", [I_DIM, BM], BF16, kind="ExternalInput")
    dinv_cols = nc.dram_tensor("dinv_cols", [P, nblk], F32, kind="ExternalInput")
    out = nc.dram_tensor("ts1", [nsh_pad, HB], BF16, kind="ExternalOutput")
    kt = I_DIM // P

    with tile.TileContext(nc) as tc:
        with (
            tc.tile_pool(name="xslab", bufs=2) as xslab_tp,
            tc.tile_pool(name="const", bufs=1) as const_tp,
            tc.tile_pool(name="psum", bufs=8, space="PSUM") as psum_tp,
            tc.tile_pool(name="stage", bufs=1) as stage_tp,
            tc.For_i(0, repeat, 1) if repeat > 1 else nullcontext(),
        ):
            w1_raw = const_tp.tile([P, kt, BM], BF16)
            nc.sync.dma_start(
                out=w1_raw[:], in_=w1.rearrange("(k p) h -> p k h", p=P)
            )
            w1_s = const_tp.tile([P, kt, BM], BF16)
            nc.vector.tensor_copy(out=w1_s[:], in_=w1_raw[:])
            dinv_s = const_tp.tile([P, nblk], F32)
            nc.sync.dma_start(out=dinv_s[:], in_=dinv_cols[:, :])
            stage = stage_tp.tile([P, nblk, HB], BF16)
            nc.vector.memset(stage[:], 0.0)

            xT_r = xT.rearrange("(k p) m -> p k m", p=P)
            for c0 in range(0, nblk, L1_MCHUNK):
                mw = min(L1_MCHUNK, nblk - c0)
                raw = xslab_tp.tile([P, kt, L1_MCHUNK * P], BF16, tag="raw")
                nc.sync.dma_start(
                    out=raw[:, :, : mw * P],
                    in_=xT_r[:, :, c0 * P : (c0 + mw) * P],
                )
                slab = xslab_tp.tile([P, kt, L1_MCHUNK * P], BF16, tag="slab")
                nc.vector.tensor_copy(
                    out=slab[:, :, : mw * P], in_=raw[:, :, : mw * P]
                )
                for m in range(mw):
                    ps = psum_tp.tile([P, BM], F32, space="PSUM")
                    for k in range(kt):
                        nc.tensor.matmul(
                            ps[:],
                            lhsT=slab[:, k, m * P : (m + 1) * P],
                            rhs=w1_s[:, k, :],
                            start=(k == 0),
                            stop=(k == kt - 1),
                        )
                    b = c0 + m
                    nc.vector.tensor_scalar_mul(
                        out=stage[:, b, :BM], in0=ps[:],
                        scalar1=dinv_s[:, b : b + 1],
                    )
            nc.sync.dma_start(
                out=out.rearrange("(b p) h -> p b h", p=P), in_=stage[:]
            )
    nc.finalize()
    return nc


def _emit_prop(nc, sched, tabs, idx_s, ind_loader, agg, msg_tp, psum_tp,
               parts="gme", on_fold=None):
    """Gather calls + indicator matmuls + PSUM->SBUF folds into agg.

    on_fold(b0_64, n_64) fires right after the FINAL chunk's bank fold, so
    per-block epilogues can interleave with the remaining gather stream
    instead of serializing after it."""
    mms, calls = sched["mms"], sched["calls"]
    last_chunk = sched["n_chunks"] - 1
    call_bounds = [(ci, c, t0, n) for ci, (c, t0, n) in enumerate(calls)]
    msg_tiles = {}
    issued = set()

    def ensure_call(ci):
        if ci in issued:
            return
        issued.add(ci)
        _, c, t0, n = call_bounds[ci]
        q = ci % NQ
        mt = msg_tp.tile([P, CALL_TOKENS // P, HB], BF16, tag=f"msg{q}")
        msg_tiles[ci] = mt
        if "g" not in parts:
            return
        nc.gpsimd.dma_gather(
            mt[:, : n // P, :],
            tabs[c],
            idx_s[:, t0 // 16 : (t0 + n) // 16],
            n,
            n,
            HB,
            single_packet=False,
            queue_num=q,
        )

    def find_call(tok0):
        for ci, c, t0, n in call_bounds:
            if t0 <= tok0 < t0 + n:
                return ci, (tok0 - t0) // P
        raise AssertionError(tok0)

    if "m" not in parts:
        for ci in range(len(call_bounds)):
            ensure_call(ci)
        return
    bank = None  # PSUM tile [BM, 8, BM]: one slot per run (64-block)
    bank_b0 = None

    def flush_bank(n_in_bank):
        bb0 = bank_b0 // 2
        ne = (n_in_bank + 1) // 2
        no = n_in_bank // 2
        nc.vector.tensor_add(
            out=agg[:BM, bb0 : bb0 + ne, :],
            in0=agg[:BM, bb0 : bb0 + ne, :],
            in1=bank[:, 0:n_in_bank:2, :],
        )
        if no:
            nc.vector.tensor_add(
                out=agg[BM:, bb0 : bb0 + no, :],
                in0=agg[BM:, bb0 : bb0 + no, :],
                in1=bank[:, 1:n_in_bank:2, :],
            )

    for i, (c, b, t0, start, stop) in enumerate(mms):
        ci, slot = find_call(t0)
        ensure_call(ci)
        ind_tile = ind_loader(i)
        if start and b % 8 == 0:
            bank = psum_tp.tile([BM, 8, BM], F32, space="PSUM", tag="agg")
            bank_b0 = b
        nc.tensor.matmul(
            bank[:, b % 8, :],
            lhsT=ind_tile,
            rhs=msg_tiles[ci][:, slot, :BM],
            start=start,
            stop=stop,
        )
        if stop and (b % 8 == 7 or i + 1 == len(mms) or mms[i + 1][0] != c):
            flush_bank(b % 8 + 1)
            if on_fold is not None and c == last_chunk:
                on_fold(bank_b0, b % 8 + 1)


def _make_ind_loader(nc, dstloc_s, iota_s, ind_tp, n_mm):
    """Generate one-hot indicator slabs on the DVE: ind[p,i,c] =
    (dstloc[p,i] == c), batched IND_G tiles per tensor_tensor(is_equal).
    Slab g+1 is generated once g is half-consumed, so its DVE op lands
    ahead of any interleaved epilogue ops and the PE never stalls on it."""
    slabs = {}

    def gen(g):
        if g in slabs or g * IND_G >= n_mm:
            return
        lo, hi = g * IND_G, min((g + 1) * IND_G, n_mm)
        ng = hi - lo
        sl = ind_tp.tile([P, IND_G, BM], BF16, tag="islab")
        nc.vector.tensor_tensor(
            out=sl[:, :ng, :],
            in0=dstloc_s[:, lo:hi].unsqueeze(2).broadcast_to([P, ng, BM]),
            in1=iota_s[:].unsqueeze(1).broadcast_to([P, ng, BM]),
            op=mybir.AluOpType.is_equal,
        )
        slabs[g] = sl

    def loader(i):
        g = i // IND_G
        gen(g)
        if i % IND_G >= IND_G // 2:
            gen(g + 1)
        return slabs[g][:, i % IND_G, :]

    return loader


def _build_l2(sched, nsh_pad, nblk, has_b1, repeat=1, parts="gme"):
    n_tab, t_tot, n_mm = sched["n_tab"], sched["t_tot"], sched["n_mm"]
    chunk_rows = sched["chunk_rows"]
    nc = bacc.Bacc(None, target_bir_lowering=False, num_swdge_queues=NQ)
    tab = nc.dram_tensor("tab", [n_tab, HB], BF16, kind="ExternalInput")
    own = nc.dram_tensor("own", [nsh_pad, HB], BF16, kind="ExternalInput")
    idx = nc.dram_tensor("idx", [P, t_tot // 16], I16, kind="ExternalInput")
    dstloc = nc.dram_tensor("dstloc", [P, n_mm], BF16, kind="ExternalInput")
    iota = nc.dram_tensor("iota", [P, BM], BF16, kind="ExternalInput")
    dinv_cols = nc.dram_tensor("dinv_cols", [P, nblk], F32, kind="ExternalInput")
    if has_b1:
        b1bc = nc.dram_tensor("b1bc", [P, BM], F32, kind="ExternalInput")
    out = nc.dram_tensor("hs", [nsh_pad, HB], BF16, kind="ExternalOutput")

    with tile.TileContext(nc) as tc:
        with (
            tc.tile_pool(name="const", bufs=1) as const_tp,
            tc.tile_pool(name="msg", bufs=3) as msg_tp,
            tc.tile_pool(name="indp", bufs=3) as ind_tp,
            tc.tile_pool(name="psum", bufs=4, space="PSUM") as psum_tp,
            tc.tile_pool(name="stage", bufs=1) as stage_tp,
            tc.For_i(0, repeat, 1) if repeat > 1 else nullcontext(),
        ):
            idx_s = const_tp.tile([P, t_tot // 16], I16)
            nc.sync.dma_start(out=idx_s[:], in_=idx[:, :])
            own_s = const_tp.tile([P, nblk, BM], BF16)
            nc.sync.dma_start(
                out=own_s[:],
                in_=own.rearrange("(b p) h -> p b h", p=P)[:, :, :BM],
            )
            dinv_s = const_tp.tile([P, nblk], F32)
            nc.sync.dma_start(out=dinv_s[:], in_=dinv_cols[:, :])
            dsq = const_tp.tile([P, nblk], F32)
            nc.vector.tensor_mul(out=dsq[:], in0=dinv_s[:], in1=dinv_s[:])
            if has_b1:
                b1_s = const_tp.tile([P, BM], F32)
                nc.sync.dma_start(out=b1_s[:], in_=b1bc[:, :])
            dstloc_s = const_tp.tile([P, n_mm], BF16)
            nc.sync.dma_start(out=dstloc_s[:], in_=dstloc[:, :])
            iota_s = const_tp.tile([P, BM], BF16)
            nc.sync.dma_start(out=iota_s[:], in_=iota[:, :])
            agg = stage_tp.tile([P, nblk, BM], F32, tag="agg")
            nc.vector.memset(agg[:], 0.0)
            stage = stage_tp.tile([P, nblk, BM], BF16, tag="out")

            loader = _make_ind_loader(nc, dstloc_s, iota_s, ind_tp, n_mm)
            tabs = {
                c: tab[c * chunk_rows : min((c + 1) * chunk_rows, n_tab), :]
                for c in range(sched["n_chunks"])
            }
            def epi(b064, n64):
                # hs = relu(agg*dinv)*dinv = relu(agg*dinv^2), 4 blocks/op
                b0, nb = b064 // 2, n64 // 2
                B = slice(b0, b0 + nb)
                nc.vector.tensor_add(
                    out=agg[:, B, :], in0=agg[:, B, :], in1=own_s[:, B, :]
                )
                nc.vector.tensor_mul(
                    out=agg[:, B, :], in0=agg[:, B, :],
                    in1=dsq[:, B].unsqueeze(2).broadcast_to([P, nb, BM]),
                )
                nc.scalar.activation(
                    out=stage[:, B, :], in_=agg[:, B, :], func=AF.Relu
                )

            use_cb = (not has_b1) and "e" in parts
            _emit_prop(nc, sched, tabs, idx_s, loader, agg, msg_tp, psum_tp,
                       parts=parts, on_fold=epi if use_cb else None)

            if has_b1 and "e" in parts:
                for b in range(nblk):
                    nc.vector.tensor_add(
                        out=agg[:, b, :], in0=agg[:, b, :], in1=own_s[:, b, :]
                    )
                    # hs = relu(agg*dinv + b1)*dinv; relu(y)*d = relu(y*d), d>0
                    nc.vector.tensor_scalar_mul(
                        out=agg[:, b, :], in0=agg[:, b, :],
                        scalar1=dinv_s[:, b : b + 1],
                    )
                    nc.vector.tensor_add(
                        out=agg[:, b, :], in0=agg[:, b, :], in1=b1_s[:]
                    )
                    nc.scalar.activation(
                        out=stage[:, b, :], in_=agg[:, b, :], func=AF.Relu,
                        scale=dinv_s[:, b : b + 1],
                    )
            nc.sync.dma_start(
                out=out.rearrange("(b p) h -> p b h", p=P)[:, :, :BM],
                in_=stage[:],
            )
    nc.finalize()
    return nc


def _build_l3(sched, nsh_pad, nblk, has_bmu, has_bvar, repeat=1):
    n_tab, t_tot, n_mm = sched["n_tab"], sched["t_tot"], sched["n_mm"]
    chunk_rows = sched["chunk_rows"]
    nc = bacc.Bacc(None, target_bir_lowering=False, num_swdge_queues=NQ)
    tab = nc.dram_tensor("tab", [n_tab, HB], BF16, kind="ExternalInput")
    own = nc.dram_tensor("own", [nsh_pad, HB], BF16, kind="ExternalInput")
    idx = nc.dram_tensor("idx", [P, t_tot // 16], I16, kind="ExternalInput")
    dstloc = nc.dram_tensor("dstloc", [P, n_mm], BF16, kind="ExternalInput")
    iota = nc.dram_tensor("iota", [P, BM], BF16, kind="ExternalInput")
    dinv_cols = nc.dram_tensor("dinv_cols", [P, nblk], F32, kind="ExternalInput")
    wmu = nc.dram_tensor("wmu", [BM, BM], F32, kind="ExternalInput")
    wvar = nc.dram_tensor("wvar", [BM, BM], F32, kind="ExternalInput")
    eps_sh = nc.dram_tensor("eps_sh", [nsh_pad, BM], BF16, kind="ExternalInput")
    if has_bmu:
        bmubc = nc.dram_tensor("bmubc", [P, BM], F32, kind="ExternalInput")
    if has_bvar:
        bvarbc = nc.dram_tensor("bvarbc", [P, BM], F32, kind="ExternalInput")
    zm = nc.dram_tensor("zm", [nsh_pad, BM], F32, kind="ExternalOutput")
    zv = nc.dram_tensor("zv", [nsh_pad, BM], F32, kind="ExternalOutput")
    zz = nc.dram_tensor("zz", [nsh_pad, BM], F32, kind="ExternalOutput")

    with tile.TileContext(nc) as tc:
        with (
            tc.tile_pool(name="const", bufs=1) as const_tp,
            tc.tile_pool(name="msg", bufs=2) as msg_tp,
            tc.tile_pool(name="indp", bufs=3) as ind_tp,
            tc.tile_pool(name="work", bufs=3) as work_tp,
            tc.tile_pool(name="psum", bufs=2, space="PSUM") as psum_tp,
            tc.tile_pool(name="stage", bufs=1) as stage_tp,
            tc.For_i(0, repeat, 1) if repeat > 1 else nullcontext(),
        ):
            idx_s = const_tp.tile([P, t_tot // 16], I16)
            nc.sync.dma_start(out=idx_s[:], in_=idx[:, :])
            own_s = const_tp.tile([P, nblk, BM], BF16)
            nc.sync.dma_start(
                out=own_s[:],
                in_=own.rearrange("(b p) h -> p b h", p=P)[:, :, :BM],
            )
            dinv_s = const_tp.tile([P, nblk], F32)
            nc.sync.dma_start(out=dinv_s[:], in_=dinv_cols[:, :])
            eps_s = const_tp.tile([P, nblk, BM], BF16)
            nc.sync.dma_start(
                out=eps_s[:], in_=eps_sh.rearrange("(b p) h -> p b h", p=P)
            )
            w_raw = const_tp.tile([P, 2, BM], F32)
            nc.sync.dma_start(out=w_raw[:BM, 0, :], in_=wmu[:, :])
            nc.sync.dma_start(out=w_raw[:BM, 1, :], in_=wvar[:, :])
            nc.sync.dma_start(out=w_raw[BM:, 0, :], in_=wmu[:, :])
            nc.sync.dma_start(out=w_raw[BM:, 1, :], in_=wvar[:, :])
            wmu_s = const_tp.tile([P, BM], F32)
            nc.vector.tensor_copy(out=wmu_s[:], in_=w_raw[:, 0, :])
            wvar_s = const_tp.tile([P, BM], F32)
            nc.vector.tensor_copy(out=wvar_s[:], in_=w_raw[:, 1, :])
            dstloc_s = const_tp.tile([P, n_mm], BF16)
            nc.sync.dma_start(out=dstloc_s[:], in_=dstloc[:, :])
            iota_s = const_tp.tile([P, BM], BF16)
            nc.sync.dma_start(out=iota_s[:], in_=iota[:, :])
            ident_raw = const_tp.tile([P, P], F32)
            make_identity(nc, ident_raw[:])
            ident = const_tp.tile([P, P], F32)
            nc.vector.tensor_copy(out=ident[:], in_=ident_raw[:])
            if has_bmu:
                bmu_s = const_tp.tile([P, BM], F32)
                nc.sync.dma_start(out=bmu_s[:], in_=bmubc[:, :])
            if has_bvar:
                bvar_s = const_tp.tile([P, BM], F32)
                nc.sync.dma_start(out=bvar_s[:], in_=bvarbc[:, :])
            agg = stage_tp.tile([P, nblk, BM], F32, tag="agg")
            nc.vector.memset(agg[:], 0.0)

            loader = _make_ind_loader(nc, dstloc_s, iota_s, ind_tp, n_mm)
            tabs = {
                c: tab[c * chunk_rows : min((c + 1) * chunk_rows, n_tab), :]
                for c in range(sched["n_chunks"])
            }
            zm_r = zm.rearrange("(b p) h -> p b h", p=P)
            zv_r = zv.rearrange("(b p) h -> p b h", p=P)
            zz_r = zz.rearrange("(b p) h -> p b h", p=P)

            def head(b064, n64):
                # heads for the 128-blocks of one fold bank (<=4 blocks),
                # emitted inside the gather stream so they overlap it
                g0, nb = b064 // 2, n64 // 2
                psm = psum_tp.tile([P, 4, BM], F32, space="PSUM", tag="psmu")
                psv = psum_tp.tile([P, 4, BM], F32, space="PSUM", tag="psvar")
                for j2 in range(nb // 2):
                    bA = g0 + 2 * j2
                    B = slice(bA, bA + 2)
                    nc.vector.tensor_add(
                        out=agg[:, B, :], in0=agg[:, B, :], in1=own_s[:, B, :]
                    )
                    p2 = work_tp.tile([P, 2, BM], F32, tag="p2")
                    nc.vector.tensor_mul(
                        out=p2[:], in0=agg[:, B, :],
                        in1=dinv_s[:, B].unsqueeze(2).broadcast_to([P, 2, BM]),
                    )
                    for k in range(2):
                        slot = 2 * j2 + k
                        pst = psum_tp.tile(
                            [BM, P], F32, space="PSUM", tag="pst"
                        )
                        nc.tensor.transpose(
                            out=pst[:], in_=p2[:, k, :], identity=ident[:]
                        )
                        p2t = work_tp.tile([BM, P], F32, tag="p2t")
                        nc.vector.tensor_copy(out=p2t[:], in_=pst[:])
                        nc.tensor.matmul(
                            psm[:, slot, :], lhsT=p2t[:],
                            rhs=wmu_s[:BM, :], start=True, stop=True,
                        )
                        nc.tensor.matmul(
                            psv[:, slot, :], lhsT=p2t[:],
                            rhs=wvar_s[:BM, :], start=True, stop=True,
                        )
                S = slice(g0, g0 + nb)
                sp = work_tp.tile([P, 4, BM], F32, tag="sp")
                if has_bvar:
                    nc.vector.tensor_add(
                        out=psv[:, :nb, :], in0=psv[:, :nb, :],
                        in1=bvar_s[:].unsqueeze(1).broadcast_to([P, nb, BM]),
                    )
                # softplus(u) = relu(u) + ln(1 + exp(-|u|))
                nc.scalar.activation(
                    out=sp[:, :nb, :], in_=psv[:, :nb, :], func=AF.Abs
                )
                nc.scalar.activation(
                    out=sp[:, :nb, :], in_=sp[:, :nb, :], func=AF.Exp,
                    scale=-1.0,
                )
                nc.scalar.activation(
                    out=sp[:, :nb, :], in_=sp[:, :nb, :], func=AF.Ln, bias=1.0
                )
                zv_t = work_tp.tile([P, 4, BM], F32, tag="zv_t")
                nc.vector.tensor_scalar_max(
                    out=zv_t[:, :nb, :], in0=psv[:, :nb, :], scalar1=0.0
                )
                nc.vector.tensor_add(
                    out=zv_t[:, :nb, :], in0=zv_t[:, :nb, :], in1=sp[:, :nb, :]
                )
                zm_t = work_tp.tile([P, 4, BM], F32, tag="zm_t")
                nc.scalar.activation(
                    out=zm_t[:, :nb, :], in_=psm[:, :nb, :], func=AF.Copy
                )
                if has_bmu:
                    nc.vector.tensor_add(
                        out=zm_t[:, :nb, :], in0=zm_t[:, :nb, :],
                        in1=bmu_s[:].unsqueeze(1).broadcast_to([P, nb, BM]),
                    )
                zz_t = work_tp.tile([P, 4, BM], F32, tag="zz_t")
                nc.vector.tensor_mul(
                    out=zz_t[:, :nb, :], in0=zv_t[:, :nb, :], in1=eps_s[:, S, :]
                )
                nc.vector.tensor_add(
                    out=zz_t[:, :nb, :], in0=zz_t[:, :nb, :], in1=zm_t[:, :nb, :]
                )
                nc.sync.dma_start(out=zm_r[:, S, :], in_=zm_t[:, :nb, :])
                nc.sync.dma_start(out=zv_r[:, S, :], in_=zv_t[:, :nb, :])
                nc.sync.dma_start(out=zz_r[:, S, :], in_=zz_t[:, :nb, :])

            _emit_prop(nc, sched, tabs, idx_s, loader, agg, msg_tp, psum_tp,
                       on_fold=head)
    nc.finalize()
    return nc


# ----------------------------------------------------------------------------
# top-level entry
# ----------------------------------------------------------------------------


def kernel(x, edge_index, W1, b1, W_mu, b_mu, W_var, b_var, eps):
    x = np.ascontiguousarray(np.asarray(x, dtype=np.float32))
    W1 = np.ascontiguousarray(np.asarray(W1, dtype=np.float32))
    W_mu = np.ascontiguousarray(np.asarray(W_mu, dtype=np.float32))
    W_var = np.ascontiguousarray(np.asarray(W_var, dtype=np.float32))
    b1 = np.asarray(b1, dtype=np.float32)
    b_mu = np.asarray(b_mu, dtype=np.float32)
    b_var = np.asarray(b_var, dtype=np.float32)
    eps = np.asarray(eps, dtype=np.float32)
    ei = np.asarray(edge_index, dtype=np.int64)

    N, I_DIM = x.shape
    H = W1.shape[1]
    assert N % M == 0 and I_DIM % P == 0 and H == BM

    src, dst = ei[0], ei[1]
    deg = (np.bincount(dst, minlength=N) + 1.0).astype(np.float32)
    dinv = (1.0 / np.sqrt(deg)).astype(np.float32)

    nsh, nsh_pad, gpos, core_of, slot_of, nodes = _permute(N, dst, src)
    sched = _schedule(src, dst, nsh_pad, gpos, core_of, slot_of)
    nblk = sched["nblk"]

    has_b1 = bool(np.any(b1 != 0))
    has_bmu = bool(np.any(b_mu != 0))
    has_bvar = bool(np.any(b_var != 0))

    xT_c, dinv_cols_c, eps_c = [], [], []
    for c in range(M):
        nl = nodes[c]
        xs = np.zeros((nsh_pad, I_DIM), dtype=np.float32)
        xs[:nsh] = x[nl]
        xT_c.append(np.ascontiguousarray(xs.T.astype(_bf16_dtype())))
        d = np.ones(nsh_pad, dtype=np.float32)
        d[:nsh] = dinv[nl]
        dinv_cols_c.append(np.ascontiguousarray(d.reshape(nblk, P).T))
        es = np.zeros((nsh_pad, H), dtype=np.float32)
        es[:nsh] = eps[nl]
        eps_c.append(es.astype(_bf16_dtype()))

    core_ids = list(range(M))
    exec_ns = []

    def _run(nc, in_maps):
        r = run_bass_kernel_spmd(nc, in_maps, core_ids, trace=PROFILE)
        if PROFILE:
            exec_ns.append(r.exec_time_ns)
        return r.results

    # ---- L1 ----
    nc1 = _build_l1(I_DIM, nsh_pad, nblk)
    W1_bf = W1.astype(_bf16_dtype())
    r1 = _run(
        nc1,
        [{"xT": xT_c[c], "w1": W1_bf, "dinv_cols": dinv_cols_c[c]} for c in range(M)],
    )
    ts1_c = [np.asarray(r1[c]["ts1"]) for c in range(M)]
    tab1 = np.ascontiguousarray(
        np.stack(ts1_c, axis=1).reshape(M * nsh_pad, HB)
    )

    # ---- L2 ----
    nc2 = _build_l2(sched, nsh_pad, nblk, has_b1)
    b1bc = np.broadcast_to(b1, (P, H)).copy() if has_b1 else None
    in_maps = []
    for c in range(M):
        im = {
            "tab": tab1,
            "own": ts1_c[c],
            "idx": sched["idx_wrapped"][c],
            "dstloc": sched["dstloc_pm"][c],
            "iota": sched["iota_row"],
            "dinv_cols": dinv_cols_c[c],
        }
        if has_b1:
            im["b1bc"] = b1bc
        in_maps.append(im)
    r2 = _run(nc2, in_maps)
    hs_c = [np.asarray(r2[c]["hs"]) for c in range(M)]
    tab2 = np.ascontiguousarray(
        np.stack(hs_c, axis=1).reshape(M * nsh_pad, HB)
    )

    # ---- L3 ----
    nc3 = _build_l3(sched, nsh_pad, nblk, has_bmu, has_bvar)
    bmubc = np.broadcast_to(b_mu, (P, H)).copy() if has_bmu else None
    bvarbc = np.broadcast_to(b_var, (P, H)).copy() if has_bvar else None
    in_maps = []
    for c in range(M):
        im = {
            "tab": tab2,
            "own": hs_c[c],
            "idx": sched["idx_wrapped"][c],
            "dstloc": sched["dstloc_pm"][c],
            "iota": sched["iota_row"],
            "dinv_cols": dinv_cols_c[c],
            "wmu": W_mu,
            "wvar": W_var,
            "eps_sh": eps_c[c],
        }
        if has_bmu:
            im["bmubc"] = bmubc
        if has_bvar:
            im["bvarbc"] = bvarbc
        in_maps.append(im)
    r3 = _run(nc3, in_maps)

    global LAST_EXEC_NS, LAST_PER_LAUNCH
    if PROFILE:
        LAST_PER_LAUNCH = exec_ns
        LAST_EXEC_NS = sum(t for t in exec_ns if t) if any(exec_ns) else None

    z_mean = np.empty((N, H), dtype=np.float32)
    z_var = np.empty((N, H), dtype=np.float32)
    z = np.empty((N, H), dtype=np.float32)
    for c in range(M):
        nl = nodes[c]
        z_mean[nl] = np.asarray(r3[c]["zm"])[:nsh]
        z_var[nl] = np.asarray(r3[c]["zv"])[:nsh]
        z[nl] = np.asarray(r3[c]["zz"])[:nsh]
    return z_mean, z_var, z

